# revision 53
# baseline (speedup 1.0000x reference)
"""MatchLSTM attention kernel for 8 Trainium2 NeuronCores.

Reference computation (B=64, T=2048, D=512):
    G   = tanh(input_p@Wp.T + bp + input_q@Wq.T + bq + h_tm1@Wr.T + br)
    a   = softmax(G@w + match_b)            over T
    z   = sum_t a[:,t] * input_q[:,:,t]
    out = concat([input_p, z], -1)

Sharding: data-parallel over batch, 8 batches per core, weights replicated.

Per-core pipeline (fp8 e4m3 matmul operands, DoubleRow perf mode = 2 k-tiles
of 128 per instruction at 0.5 cycles/col, fp32 PSUM accumulation):
  - c^T[o,b] = (Wp.T;Wr.T;bias) @ (ip.T;h.T;1)  [once, fp8 DoubleRow]
  - X^T tiles [q,tok] and X natural tiles [tok,q] DMA'd fp8 (host provides
    both layouts pre-transposed; no DMA-transpose needed)
  - G^T[o,tok] = Wq.T @ X^T (fp8 DoubleRow), tanh on ScalarE with
    per-partition bias c^T -> fp8 SBUF
  - scores transposed for free: sT[tok,1] = th-chunk (stationary) @ wcol
    (moving, N=1); w is scaled by 32 host-side to stay normal in fp8
  - exp(sT/32 - 1.5) on ScalarE -> fp8 esc columns (match_b cancels in
    softmax; the -1.5 shift keeps exp well within fp8 range and cancels too)
  - z^T[q,1] += xnat-chunk (stationary) @ esc pair (moving, N=1), and
    sumexp  += esc pair (stationary) @ ones (consistent with quantized esc)
  - z^T and sumexp DMA'd out unnormalized; host divides and re-lays out.
Scores/z/sumexp matmuls move N=1 columns so their PE cost is ~nil; PE time
is the G matmul; ScalarE tanh is the bottleneck. PE consumers of ScalarE
outputs are emitted a few G-stages late so the in-order PE queue never
parks on a ScalarE dependency.
"""

import sys

if "/opt/trn_rl_repo" not in sys.path:
    sys.path.insert(0, "/opt/trn_rl_repo")

import numpy as np
import ml_dtypes

N_CORES = 8
B, T, D = 64, 2048, 512
PB = B // N_CORES          # batches per core
KC = D // 128              # 4 contraction chunks of 128
NJ = T // 128              # 16 token chunks of 128
CROWS = 1280               # cw/cx rows: Wp.T, Wr.T, bias row, zero pad (10*128)
NKP = CROWS // 256         # 5 DoubleRow pairs for the c matmul

F8 = ml_dtypes.float8_e4m3
WSCALE = 32.0
ESHIFT = -1.5

_CACHE: dict = {}


def _build_program():
    import concourse.bacc as bacc
    import concourse.tile as tile
    import concourse.mybir as mybir
    from concourse.bass import MemorySpace

    dt = mybir.dt
    F32 = dt.float32
    FP8 = dt.float8e4
    BF = dt.bfloat16
    AF = mybir.ActivationFunctionType
    DR = mybir.MatmulPerfMode.DoubleRow

    nc = bacc.Bacc(
        "TRN2", target_bir_lowering=False, debug=False, num_devices=N_CORES
    )

    xqt_d = nc.dram_tensor("xqt", [PB, D, T], FP8, kind="ExternalInput")
    xq_d = nc.dram_tensor("xq", [PB, T, D], FP8, kind="ExternalInput")
    # Wq.T re-laid oc-major: row oc*128+p holds Wq.T[kc*128+p, oc*128:+128]
    # for all kc, so per-oc chunks DMA contiguously.
    wq_d = nc.dram_tensor("wqt", [KC * 128, KC * 128], FP8, kind="ExternalInput")
    # cw re-laid per oc-chunk: row oc*128+p holds cw[(k*2+two)*128+p, oc*128:+128]
    # for all (k,two) — so each oc's DMA moves contiguous 1280B lines.
    cw_d = nc.dram_tensor("cw", [KC * 128, NKP * 2 * 128], FP8, kind="ExternalInput")
    cx_d = nc.dram_tensor("cx", [CROWS, PB], FP8, kind="ExternalInput") # [ip.T;h.T;1;0]
    wcol_d = nc.dram_tensor("wcol", [D, 1], BF, kind="ExternalInput")   # 32*w
    z_d = nc.dram_tensor("z", [128, PB * (KC + 1)], F32, kind="ExternalOutput")

    with tile.TileContext(nc) as tc:
        with (
            tc.tile_pool(name="consts", bufs=1) as consts,
            tc.tile_pool(name="xT_p", bufs=3) as xT_pool,
            tc.tile_pool(name="xnat_p", bufs=3) as xnat_pool,
            tc.tile_pool(name="tanh_p", bufs=3) as tanh_pool,
            tc.tile_pool(name="dve_p", bufs=2) as dve_pool,
            tc.tile_pool(name="esc_p", bufs=2) as esc_pool,
            tc.tile_pool(name="out_p", bufs=1) as out_pool,
            # PSUM budget (8 banks of 2KB/partition):
            #   pG 3 bufs x [128,1024]f32 (2 banks each)             = 6 banks
            #   pSml 2 bufs x 1 bank (scores sT + z^T + sumexp, all
            #   of one batch share a single bank; c_ps at setup)     = 2 banks
            tc.tile_pool(name="pG", bufs=3, space=MemorySpace.PSUM) as pG,
            tc.tile_pool(name="pSml", bufs=2, space=MemorySpace.PSUM) as pSml,
        ):
            # ---- constants -------------------------------------------------
            # Startup latency: spread the critical first DMAs across the SP /
            # DVE / Act DGE queues and split big transfers so the first G
            # matmul and the first bias chunk land as early as possible.
            wq_s = consts.tile([128, KC, KC, 128], FP8, tag="wq", name="wq_s")
            nc.sync.dma_start(
                out=wq_s[:, 0:2, :, :],
                in_=wq_d[0:256, :].rearrange("(oc p) (c o) -> p oc c o", p=128, c=KC),
            )

            xT0 = xT_pool.tile([128, KC, T], FP8, tag="xT", name="xT")
            nc.scalar.dma_start(
                out=xT0[:, :, 0:512],
                in_=xqt_d[0, :, 0:512].rearrange("(c p) t -> p c t", p=128),
            )
            nc.scalar.dma_start(
                out=xT0[:, :, 512:1024],
                in_=xqt_d[0, :, 512:1024].rearrange("(c p) t -> p c t", p=128),
            )

            cw_s = consts.tile([128, KC, NKP, 2, 128], FP8, tag="cw", name="cw_s")
            cx_s = consts.tile([128, NKP, 2, PB], FP8, tag="cx", name="cx_s")

            def cw_dma(oc):
                nc.sync.dma_start(
                    out=cw_s[:, oc, :, :, :],
                    in_=cw_d[oc * 128 : (oc + 1) * 128, :].rearrange(
                        "p (c two o) -> p c two o", c=NKP, two=2
                    ),
                )

            cw_dma(0)
            nc.sync.dma_start(
                out=cx_s, in_=cx_d.rearrange("(c two p) b -> p c two b", p=128, two=2)
            )
            cw_dma(1)
            cw_dma(2)
            cw_dma(3)
            nc.sync.dma_start(
                out=wq_s[:, 2:4, :, :],
                in_=wq_d[256:512, :].rearrange("(oc p) (c o) -> p oc c o", p=128, c=KC),
            )
            wcol_s = consts.tile([128, KC, 1], BF, tag="wcol", name="wcol_s")
            nc.sync.dma_start(out=wcol_s, in_=wcol_d.rearrange("(c p) o -> p c o", p=128))

            nc.sync.dma_start(
                out=xT0[:, :, 1024:2048],
                in_=xqt_d[0, :, 1024:2048].rearrange("(c p) t -> p c t", p=128),
            )

            ones8 = consts.tile([128, 2, 1], FP8, tag="ones8", name="ones8")
            nc.vector.memset(ones8, 1.0)
            eb_s = consts.tile([128, 1], F32, tag="eb", name="eb_s")
            nc.vector.memset(eb_s, ESHIFT)
            acc_junk = consts.tile([128, 1], F32, tag="ajunk", name="acc_junk")

            # c^T[o, b] computed just-in-time per oc during batch 0's first
            # stages (emit_c below) so the cw DMAs don't block early G work.
            # PSUM `start` marks the whole 2KB bank pending-zero, so exactly
            # one start per bank: the first instruction touching it.
            c_ps = pSml.tile([128, KC, PB], F32, tag="sml", name="c_ps")
            cT_s = consts.tile([128, KC, PB], F32, tag="cT", name="cT_s")

            def emit_c(oc):
                for k in range(NKP):
                    nc.tensor.matmul(
                        c_ps[:, oc, :],
                        cw_s[:, oc, k, :, :],
                        cx_s[:, k, :, :],
                        start=(oc == 0 and k == 0),
                        stop=(oc == KC - 1 and k == NKP - 1),
                        perf_mode=DR,
                        skip_group_check=True,
                    )
                # ScalarE, not DVE: the DVE queue is busy with the Padé tanh
                # and would delay the bias far past the first ScalarE tanh
                # (ScalarE is idle during startup; Pool can't read PSUM).
                nc.scalar.copy(out=cT_s[:, oc, :], in_=c_ps[:, oc, :])

            zts = out_pool.tile([128, PB, KC + 1], F32, tag="zts", name="zts")

            # ---- per-batch software-pipelined loop ------------------------
            pending: list = []  # (due_global_stage, seq, fn)
            gidx = 0
            seq = 0

            state: dict = {}

            def flush(upto):
                nonlocal pending
                ready = sorted((p for p in pending if p[0] <= upto))
                pending = [p for p in pending if p[0] > upto]
                for _, _, fn in ready:
                    fn()

            def defer(due, fn):
                nonlocal seq
                pending.append((due, seq, fn))
                seq += 1

            # sml bank layout per batch: [0:8] sT(tp0), [8:16] sT(tp1),
            # [16:20] z^T cols, [20] sumexp. One bank -> one start=True:
            # the first score matmul of tp0.
            def emit_pade(g_ap, th_ap, bias_ap, cols):
                # tanh(x) ~ x*(1/9 + (8/3)/(3+x^2)); equals +-1 exactly at
                # x=+-3, so clamping the result to [-1,1] handles saturation.
                AT = mybir.AluOpType
                gb = dve_pool.tile([128, 1024], BF, tag="gb", name="gb")[:, 0:cols]
                nc.vector.tensor_scalar(
                    out=gb, in0=g_ap, scalar1=bias_ap, scalar2=None, op0=AT.add
                )
                u = dve_pool.tile([128, 1024], BF, tag="u", name="u")[:, 0:cols]
                nc.vector.tensor_tensor(out=u, in0=gb, in1=gb, op=AT.mult)
                v = dve_pool.tile([128, 1024], BF, tag="v", name="v")[:, 0:cols]
                nc.vector.tensor_scalar_add(out=v, in0=u, scalar1=3.0)
                r = dve_pool.tile([128, 1024], BF, tag="r", name="r")[:, 0:cols]
                with nc.allow_low_precision(reason="pade tanh rcp"):
                    nc.vector.reciprocal(out=r, in_=v)
                y = dve_pool.tile([128, 1024], BF, tag="y", name="y")[:, 0:cols]
                nc.vector.affine_mul_reduce(
                    out=y, accum_out=acc_junk, in0=r, in1=gb,
                    scale=8.0 / 3.0, bias=1.0 / 9.0,
                )
                nc.vector.tensor_scalar(
                    out=th_ap, in0=y, scalar1=-1.0, scalar2=1.0,
                    op0=AT.max, op1=AT.min,
                )

            def emit_scores(b, tp):
                st = state[b]
                th = st["th"][tp]
                sml = st["sml"]
                # th is bf16, so plain N=1 matmuls (out-free cost ~0 either way)
                for tc_ in range(8):
                    for oc in range(KC):
                        nc.tensor.matmul(
                            sml[:, tp * 8 + tc_ : tp * 8 + tc_ + 1],
                            th[:, oc, tc_ * 128 : (tc_ + 1) * 128],
                            wcol_s[:, oc, :],
                            start=(tp == 0 and tc_ == 0 and oc == 0),
                            stop=False,
                            skip_group_check=True,
                        )

            def emit_exp(b):
                st = state[b]
                nc.scalar.activation(
                    out=st["esc"].rearrange("p j one -> p (j one)"),
                    in_=st["sml"][:, 0:NJ],
                    func=AF.Exp,
                    bias=eb_s,
                    scale=1.0 / WSCALE,
                )

            def emit_z(b):
                st = state[b]
                esc, xnat, sml = st["esc"], st["xnat"], st["sml"]
                for jp in range(8):
                    for qc in range(KC):
                        nc.tensor.matmul(
                            sml[:, 16 + qc : 16 + qc + 1],
                            xnat[:, 2 * jp : 2 * jp + 2, qc * 128 : (qc + 1) * 128],
                            esc[:, 2 * jp : 2 * jp + 2, :],
                            start=False,
                            stop=False,
                            perf_mode=DR,
                            skip_group_check=True,
                        )
                    for j in (2 * jp, 2 * jp + 1):
                        nc.tensor.matmul(
                            sml[0:1, 20:21],
                            esc[:, j, :],
                            ones8[:, 0, :],
                            start=False,
                            stop=(j == NJ - 1),
                            skip_group_check=True,
                        )

            def emit_out(b):
                st = state[b]
                nc.vector.tensor_copy(out=zts[:, b, 0:KC], in_=st["sml"][:, 16:20])
                nc.vector.tensor_copy(
                    out=zts[0:1, b, KC : KC + 1], in_=st["sml"][0:1, 20:21]
                )

            xT_tiles = {0: xT0}
            for b in range(PB):
                # prefetch next batch's X^T one full batch ahead; the natural
                # copy (needed a batch later, by emit_z) is queued mid-batch.
                if b + 1 < PB:
                    xT_tiles[b + 1] = xT_pool.tile(
                        [128, KC, T], FP8, tag="xT", name="xT"
                    )
                    nc.sync.dma_start(
                        out=xT_tiles[b + 1],
                        in_=xqt_d[b + 1].rearrange("(c p) t -> p c t", p=128),
                    )
                xT = xT_tiles[b]
                xnat = xnat_pool.tile([128, NJ, D], FP8, tag="xnat", name="xnat")
                state[b] = {
                    "xnat": xnat,
                    "th": {},
                    "esc": esc_pool.tile([128, NJ, 1], FP8, tag="esc", name="esc"),
                    "sml": pSml.tile([128, 24], F32, tag="sml", name="sml_ps"),
                }

                for tp in range(2):
                    if tp == 1:
                        nc.sync.dma_start(
                            out=xnat, in_=xq_d[b].rearrange("(j p) q -> p j q", p=128)
                        )
                    th = tanh_pool.tile([128, KC, 1024], BF, tag="th", name="th")
                    state[b]["th"][tp] = th
                    for oc in range(KC):
                        g_ps = pG.tile([128, 1024], F32, tag="g", name="g_ps")
                        for h in range(2):          # one 2KB bank per h
                            for kg in range(2):
                                for i in range(2):
                                    nc.tensor.matmul(
                                        g_ps[:, h * 512 + i * 256 : h * 512 + (i + 1) * 256],
                                        wq_s[:, oc, 2 * kg : 2 * kg + 2, :],
                                        xT[:, 2 * kg : 2 * kg + 2,
                                           tp * 1024 + h * 512 + i * 256 :
                                           tp * 1024 + h * 512 + (i + 1) * 256],
                                        start=(kg == 0 and i == 0),
                                        stop=(kg == 1 and i == 1),
                                        perf_mode=DR,
                                        skip_group_check=True,
                                    )
                        if b == 0 and tp == 0:
                            emit_c(oc)
                        # tanh: (tp0,oc0) fully and (tp1,oc0) half offloaded
                        # to DVE (Padé(3,2), final clamp on the Pool engine);
                        # the rest on ScalarE.
                        dve_cols = 1024 if (tp == 0 and oc == 0) else (
                            512 if (tp == 1 and oc == 0) else 0
                        )
                        if dve_cols:
                            lo = 1024 - dve_cols
                            emit_pade(
                                g_ps[:, lo:1024],
                                th[:, 0, lo:1024],
                                cT_s[:, oc, b : b + 1],
                                dve_cols,
                            )
                        if dve_cols < 1024:
                            nc.scalar.activation(
                                out=th[:, oc, 0 : 1024 - dve_cols],
                                in_=g_ps[:, 0 : 1024 - dve_cols],
                                func=AF.Tanh,
                                bias=cT_s[:, oc, b : b + 1],
                                scale=1.0,
                            )
                        gidx += 1
                        flush(gidx)

                    # schedule this tp's consumers into the future stream
                    # (batch 0's pipeline starts later: give it one extra stage)
                    bb, tt = b, tp
                    off = 1 if b == 0 else 0
                    defer(gidx + 3 + off, lambda bb=bb, tt=tt: emit_scores(bb, tt))
                    if tp == 1:
                        defer(gidx + 3 + off, lambda bb=bb: emit_exp(bb))
                        defer(gidx + 5 + off, lambda bb=bb: emit_z(bb))
                        defer(gidx + 6 + off, lambda bb=bb: emit_out(bb))

            # drain the pipeline tail
            flush(10**9)

            nc.sync.dma_start(out=z_d[:, :], in_=zts.rearrange("p b c -> p (b c)"))

    nc.compile()
    return nc


def _get_program():
    if "nc" not in _CACHE:
        _CACHE["nc"] = _build_program()
    return _CACHE["nc"]


def kernel(**inputs) -> np.ndarray:
    from concourse import bass_utils

    inp = {k: np.asarray(v) for k, v in inputs.items()}
    input_p = inp["input_p"].astype(np.float32)
    input_q = inp["input_q"].astype(np.float32)
    h_tm1 = inp["h_tm1"].astype(np.float32)
    Wp, Wq, Wr = inp["Wp"], inp["Wq"], inp["Wr"]
    bp, bq, br = inp["bp"], inp["bq"], inp["br"]
    w = inp["w"]

    # shared (weight) tensors
    wqt = np.asarray(Wq, np.float32).T.astype(F8)       # [q, o]
    # re-lay oc-major: row oc*128+p holds wqt[kc*128+p, oc*128:+128] for all kc
    wqh = np.ascontiguousarray(
        wqt.reshape(KC, 128, KC, 128).transpose(2, 1, 0, 3).reshape(D, D)
    )
    cw = np.zeros((CROWS, D), dtype=F8)
    cw[:D] = np.asarray(Wp, np.float32).T.astype(F8)
    cw[D : 2 * D] = np.asarray(Wr, np.float32).T.astype(F8)
    cw[2 * D] = (
        np.asarray(bp, np.float32) + np.asarray(bq, np.float32) + np.asarray(br, np.float32)
    ).astype(F8)
    # re-lay to [oc*128+p, (k,two)*128+o'] so each oc chunk DMAs contiguously
    cwr = cw.reshape(NKP * 2, 128, KC, 128)          # [k2, p, oc, o']
    cwh = np.ascontiguousarray(
        cwr.transpose(2, 1, 0, 3).reshape(KC * 128, NKP * 2 * 128)
    )
    wcol = np.ascontiguousarray(
        (WSCALE * np.asarray(w, np.float32)).reshape(D, 1)
    ).astype(ml_dtypes.bfloat16)

    nc = _get_program()

    in_maps = []
    for c in range(N_CORES):
        s = slice(c * PB, (c + 1) * PB)
        cx = np.zeros((CROWS, PB), dtype=F8)
        cx[:D] = input_p[s].T.astype(F8)
        cx[D : 2 * D] = h_tm1[s].T.astype(F8)
        cx[2 * D] = 1.0
        in_maps.append(
            {
                "xqt": np.ascontiguousarray(input_q[s].transpose(0, 2, 1)).astype(F8),
                "xq": np.ascontiguousarray(input_q[s]).astype(F8),
                "wqt": wqh,
                "cw": cwh,
                "cx": cx,
                "wcol": wcol,
            }
        )

    res = bass_utils.run_bass_kernel_spmd(
        nc, in_maps, core_ids=list(range(N_CORES))
    )
    zs = []
    for c in range(N_CORES):
        arr = np.asarray(res.results[c]["z"], dtype=np.float32).reshape(128, PB, KC + 1)
        se = arr[0, :, KC]
        # z[b, qc*128 + p] = arr[p, b, qc] / se[b]
        zs.append(arr[:, :, :KC].transpose(1, 2, 0).reshape(PB, D) / se[:, None])
    z = np.concatenate(zs, axis=0)
    return np.concatenate([input_p, z], axis=1)
